# revision 2
# baseline (speedup 1.0000x reference)
"""CTC alignment distillation loss on 8 Trainium2 NeuronCores.

Strategy ("v4", frame-balanced data-parallel, fp8, PE-Frobenius):
  * Only non-blank frames contribute (~2.4k of B*T=8192 positions).  All
    index math (frame mask, run ids `lm`, label gather y_t, per-frame
    weights w_j = 1/(B*n_b)) is tiny [B,T] integer work done on host.
  * Frames are split per-frame (not per-sequence) across the 8 cores:
    each core gets ceil(NJ/8) frames -> perfectly balanced tiles.
  * Per-frame loss contribution (pre-weight):
        contrib_j = sum_v slab'_j[v]*r_j[v] + C_Y*ry_j - lse_j
    where slab' = W*soft + (1-W)*A_R folds the label-smoothing sum-term
    (v3 trick), ry = logits[b,t,y_t] (host-gathered), lse = log sum exp r.
  * NEW in v4: the per-frame weights w_j are folded into the slab too, so
    the soft part collapses to ONE global Frobenius inner product
        G = sum_j sum_v slabW_j[v] * r_j[v],   slabW = w*slab'*SCALE
    which the PE computes directly as the accumulated diagonal of
    stationary(slab chunk)^T x moving(logits chunk) over 125-col chunks
    into a single PSUM bank -- no DVE elementwise product needed at all.
  * Esum_j on ACT: one Exp activation per tile with accum_out (per-lane
    free-axis sum).  Tail frames are V-split over `s` lanes so the last
    tile stays at fd=8000/s; host adds the s partial Esums and takes log.
  * Both operand slabs are packed host-side as dense [128, FDtot] fp8e4m3
    (TRN flavor, max +-240).  DMA per core: 2 x 2.5MB plain streams.
  * Engine budget per core/pass (NJ/core ~ 299):  ACT exp ~17.5us (bound),
    PE ~13us (160 MMs, fp8+FWL), DMA ~14us, DVE ~0.3us (PSUM evacuate).
"""

import numpy as np
from contextlib import ExitStack

B, T, V = 16, 512, 8000
BLANK = 0
LSM = 0.1
W_SOFT = 0.5
N_CORES = 8
P = 128
CHUNK = 125  # PE chunk width; divides 8000/s for s in {1,2,4,8,16}
SCALE = 2.0**23  # fp8 range centering for the weighted soft slab

A_Y = (1.0 - LSM) - LSM / (V - 1)
A_R = LSM / (V - 1)
C_Y = (1.0 - W_SOFT) * A_Y

_PROGRAM_CACHE: dict = {}


def _geometry(njmax: int) -> tuple:
    """Tiles of (nframes, vsplit, fd, off) covering njmax frames.

    Full-V tiles of up to 128 frames; a short tail (<=64 frames) is
    V-split over s lanes/frame so its activation pass is only 8000/s wide.
    """
    tiles = []
    rest, off = njmax, 0
    while rest > 0:
        if rest > 64:
            n, s = min(rest, P), 1
        else:
            s = 1
            while s < 16 and rest * s * 2 <= P:
                s *= 2
            n = rest
        fd = V // s
        tiles.append((n, s, fd, off))
        off += fd
        rest -= n
    return tuple(tiles), off


def _build_program(TS: tuple, reps: int = 1, variant: str = "v4", loop_reps: int = 0):
    """Bass/Tile program for tile geometry TS (from _geometry).

    variant: "v4"      - full kernel
             "v4dma"   - DMA only (roofline probe)
             "v4nodma" - compute only (operands DMA'd once, resident)
             "v4pe"    - DMA + PE Frobenius only
             "v4act"   - DMA + ACT exp/accum only
    reps/loop_reps: body repetition (python-unrolled / hardware For_i)
    for steady-state timing probes.
    """
    import concourse.tile as tile
    from concourse import bacc, mybir

    f32 = mybir.dt.float32
    fp8 = mybir.dt.float8e4
    bf16 = mybir.dt.bfloat16
    ACTF = mybir.ActivationFunctionType

    NT = len(TS)
    FDtot = TS[-1][3] + TS[-1][2]
    NCH = FDtot // CHUNK
    assert FDtot % CHUNK == 0
    do_dma = variant != "v4nodma"
    do_pe = variant in ("v4", "v4nodma", "v4pe")
    do_act = variant in ("v4", "v4nodma", "v4act")

    nc = bacc.Bacc(
        "TRN2", target_bir_lowering=False, debug=False, num_devices=N_CORES
    )
    lg_d = nc.dram_tensor("lg", [P, FDtot], fp8, kind="ExternalInput")
    slab_d = nc.dram_tensor("slab", [P, FDtot], fp8, kind="ExternalInput")
    stats_d = nc.dram_tensor("stats", [P, NT + P], f32, kind="ExternalOutput")

    with tile.TileContext(nc) as tc, ExitStack() as ctx:
        iopool = ctx.enter_context(tc.tile_pool(name="io", bufs=2))
        spool = ctx.enter_context(tc.tile_pool(name="small", bufs=2))
        pspool = ctx.enter_context(tc.tile_pool(name="ps", bufs=2, space="PSUM"))
        fixed = ctx.enter_context(tc.tile_pool(name="fixed", bufs=1))
        scratch = fixed.tile([P, V], bf16)  # exp elementwise dump (reused)
        if not do_dma:
            lg_res = fixed.tile([P, FDtot], fp8)
            slab_res = fixed.tile([P, FDtot], fp8)
            nc.sync.dma_start(lg_res[:], lg_d.ap())
            nc.sync.dma_start(slab_res[:], slab_d.ap())

        def emit_pass():
            if do_dma:
                lg = iopool.tile([P, FDtot], fp8)
                slab = iopool.tile([P, FDtot], fp8)
                nc.sync.dma_start(lg[:], lg_d.ap())
                nc.sync.dma_start(slab[:], slab_d.ap())
            else:
                lg, slab = lg_res, slab_res
            stats = spool.tile([P, NT + P], f32)
            if do_pe:
                ps = pspool.tile([P, 512], f32)
                for c in range(NCH):
                    nc.tensor.matmul(
                        ps[:CHUNK, :CHUNK],
                        slab[:, CHUNK * c : CHUNK * (c + 1)],
                        lg[:, CHUNK * c : CHUNK * (c + 1)],
                        start=(c == 0),
                        stop=(c == NCH - 1),
                    )
                nc.vector.tensor_copy(stats[:, NT:], ps[:, :P])
            else:
                nc.any.memset(stats[:, NT:], 0.0)
            if do_act:
                for i, (n, s, fd, off) in enumerate(TS):
                    nc.scalar.activation(
                        out=scratch[:, :fd],
                        in_=lg[:, off : off + fd],
                        func=ACTF.Exp,
                        accum_out=stats[:, i : i + 1],
                    )
            else:
                nc.any.memset(stats[:, :NT], 1.0)
            nc.sync.dma_start(stats_d.ap(), stats[:])

        if loop_reps:
            with tc.For_i(0, loop_reps, 1):
                for _ in range(reps):
                    emit_pass()
        else:
            for _ in range(reps):
                emit_pass()

    nc.compile()
    return nc


def _host_prep(ys, aligns, xlens):
    """Mirror of the reference's index math -> global non-blank frame list."""
    frame_mask = np.arange(T)[None, :] < xlens[:, None]
    a = np.where(frame_mask, aligns, BLANK)
    nonblank = a != BLANK
    shifted = np.concatenate([np.full((B, 1), BLANK, a.dtype), a[:, :-1]], axis=1)
    run_start = nonblank & (a != shifted)
    label_id = np.cumsum(run_start.astype(np.int64), axis=1) - 1
    lm = np.maximum(label_id, 0)
    n_exists = nonblank.sum(axis=1)

    bb, tt = np.nonzero(nonblank)
    w = 1.0 / (B * n_exists[bb].astype(np.float64))
    lmf = lm[bb, tt]
    y_t = np.asarray(ys)[bb, lmf]
    return bb, tt, lmf, y_t, w


def prepare(inputs: dict, variant: str = "v4"):
    """Host prep: index math, frame balancing, fp8 slab packing."""
    import ml_dtypes

    fp8np = ml_dtypes.float8_e4m3

    logits = np.asarray(inputs["logits"], dtype=np.float32)
    soft = np.asarray(inputs["soft_labels"], dtype=np.float32)
    ys = np.asarray(inputs["ys"])
    aligns = np.asarray(inputs["aligns"])
    xlens = np.asarray(inputs["xlens"])

    bb, tt, lmf, y_t, w = _host_prep(ys, aligns, xlens)
    NJ_tot = len(bb)
    ry = logits[bb, tt, y_t].astype(np.float64)

    q, r = divmod(NJ_tot, N_CORES)
    counts = [q + 1] * r + [q] * (N_CORES - r)
    njmax = q + (1 if r else 0)
    TS, FDtot = _geometry(njmax)

    key = (TS, variant)
    nc = _PROGRAM_CACHE.get(key)
    if nc is None:
        nc = _build_program(TS, variant=variant)
        _PROGRAM_CACHE[key] = nc

    starts = np.cumsum([0] + counts)
    in_maps, cores = [], []
    for c in range(N_CORES):
        s0, s1 = int(starts[c]), int(starts[c + 1])
        n = s1 - s0
        lg_rows = logits[bb[s0:s1], tt[s0:s1]]  # [n, V] f32
        soft_rows = soft[bb[s0:s1], lmf[s0:s1]]  # [n, V] f32
        slab_rows = (W_SOFT * soft_rows + (1.0 - W_SOFT) * A_R) * (
            w[s0:s1, None] * SCALE
        ).astype(np.float32)

        lg_flat = np.zeros((P, FDtot), np.float32)
        slab_flat = np.zeros((P, FDtot), np.float32)
        pos = 0
        for nf, s, fd, off in TS:
            m = min(nf, n - pos)
            if m > 0:
                lg_flat[: m * s, off : off + fd] = lg_rows[pos : pos + m].reshape(
                    m * s, fd
                )
                slab_flat[: m * s, off : off + fd] = slab_rows[
                    pos : pos + m
                ].reshape(m * s, fd)
            pos += nf
        in_maps.append(
            {"lg": lg_flat.astype(fp8np), "slab": slab_flat.astype(fp8np)}
        )
        cores.append(dict(n=n, w=w[s0:s1], ry=ry[s0:s1]))
    return nc, in_maps, cores, counts, TS


def combine(results, cores, NJ, TS, variant: str = "v4") -> np.float32:
    """Fold per-core [P, NT+P] stats into the scalar loss."""
    NT = len(TS)
    total = 0.0
    for c, ci in enumerate(cores):
        st = np.asarray(results[c]["stats"], dtype=np.float64)
        n = ci["n"]
        es = np.empty(n, np.float64)
        pos = 0
        for i, (nf, s, fd, off) in enumerate(TS):
            m = min(nf, n - pos)
            if m <= 0:
                break
            col = st[:, i]
            if s == 1:
                es[pos : pos + m] = col[:m]
            else:
                es[pos : pos + m] = col[: m * s].reshape(m, s).sum(axis=1)
            pos += nf
        lse = np.log(es)
        total += np.trace(st[:, NT:]) / SCALE
        total += float((ci["w"] * (C_Y * ci["ry"] - lse)).sum())
    return np.float32(-total)


def run(inputs: dict, variant: str = "v4", trace: bool = False, trace_cores=None):
    from concourse.bass_utils import run_bass_kernel_spmd

    nc, in_maps, cores, NJ, TS = prepare(inputs, variant)
    res = run_bass_kernel_spmd(
        nc,
        in_maps,
        list(range(N_CORES)),
        trace=trace,
        trace_cores=trace_cores,
    )
    loss = combine(res.results, cores, NJ, TS, variant)
    return loss, res


def kernel(**inputs) -> np.ndarray:
    loss, _ = run(inputs)
    return np.asarray(loss, dtype=np.float32)


# revision 4
# speedup vs baseline: 1.0864x; 1.0864x over previous
"""CTC alignment distillation loss on 8 Trainium2 NeuronCores.

Strategy ("v4", frame-balanced data-parallel, fp8, PE-Frobenius):
  * Only non-blank frames contribute (~2.4k of B*T=8192 positions).  All
    index math (frame mask, run ids `lm`, label gather y_t, per-frame
    weights w_j = 1/(B*n_b)) is tiny [B,T] integer work done on host.
  * Frames are split per-frame (not per-sequence) across the 8 cores:
    each core gets ceil(NJ/8) frames -> perfectly balanced tiles.
  * Per-frame loss contribution (pre-weight):
        contrib_j = sum_v slab'_j[v]*r_j[v] + C_Y*ry_j - lse_j
    where slab' = W*soft + (1-W)*A_R folds the label-smoothing sum-term
    (v3 trick), ry = logits[b,t,y_t] (host-gathered), lse = log sum exp r.
  * NEW in v4: the per-frame weights w_j are folded into the slab too, so
    the soft part collapses to ONE global Frobenius inner product
        G = sum_j sum_v slabW_j[v] * r_j[v],   slabW = w*slab'*SCALE
    which the PE computes directly as the accumulated diagonal of
    stationary(slab chunk)^T x moving(logits chunk) over 125-col chunks
    into a single PSUM bank -- no DVE elementwise product needed at all.
  * Esum_j on ACT: one Exp activation per tile with accum_out (per-lane
    free-axis sum).  Tail frames are V-split over `s` lanes so the last
    tile stays at fd=8000/s; host adds the s partial Esums and takes log.
  * Both operand slabs are packed host-side as dense [128, FDtot] fp8e4m3
    (TRN flavor, max +-240).  DMA per core: 2 x 2.5MB plain streams.
  * Engine budget per core/pass (NJ/core ~ 299):  ACT exp ~17.5us (bound),
    PE ~13us (160 MMs, fp8+FWL), DMA ~14us, DVE ~0.3us (PSUM evacuate).
"""

import numpy as np
from contextlib import ExitStack

B, T, V = 16, 512, 8000
BLANK = 0
LSM = 0.1
W_SOFT = 0.5
N_CORES = 8
P = 128
CHUNK = 125  # PE chunk width; divides 8000/s for s in {1,2,4,8,16}
SCALE = 2.0**23  # fp8 range centering for the weighted soft slab

A_Y = (1.0 - LSM) - LSM / (V - 1)
A_R = LSM / (V - 1)
C_Y = (1.0 - W_SOFT) * A_Y

_PROGRAM_CACHE: dict = {}


def _tail_plan(rest: int, memo: dict) -> tuple:
    """DP over the <=64-frame tail: minimize ACT cycles (sum fd + 352/tile).

    Choices: place min(rest, 128//s) frames V-split over s lanes each.
    Returns (cost, tiles) with tiles = ((n, s), ...).
    """
    if rest == 0:
        return 0, ()
    if rest in memo:
        return memo[rest]
    best = None
    for s in (2, 4, 8, 16):
        n = min(rest, P // s)
        c_rest, t_rest = _tail_plan(rest - n, memo)
        c = V // s + 352 + c_rest
        if best is None or c < best[0]:
            best = (c, ((n, s),) + t_rest)
    memo[rest] = best
    return best


def _geometry(njmax: int) -> tuple:
    """Tiles of (nframes, vsplit, fd, off) covering njmax frames.

    Full-V tiles of up to 128 frames; the short tail (<=64 frames) is
    V-split over s lanes/frame (DP-chosen) so ACT work stays near the
    njmax*V/128 floor.
    """
    tiles = []
    rest, off = njmax, 0
    while rest > 64:
        n = min(rest, P)
        tiles.append((n, 1, V, off))
        off += V
        rest -= n
    if rest:
        _, tail = _tail_plan(rest, {})
        for n, s in tail:
            fd = V // s
            tiles.append((n, s, fd, off))
            off += fd
    return tuple(tiles), off


def _build_program(TS: tuple, reps: int = 1, variant: str = "v4", loop_reps: int = 0):
    """Bass/Tile program for tile geometry TS (from _geometry).

    variant: "v4"      - full kernel
             "v4dma"   - DMA only (roofline probe)
             "v4nodma" - compute only (operands DMA'd once, resident)
             "v4pe"    - DMA + PE Frobenius only
             "v4act"   - DMA + ACT exp/accum only
    reps/loop_reps: body repetition (python-unrolled / hardware For_i)
    for steady-state timing probes.
    """
    import concourse.tile as tile
    from concourse import bacc, mybir

    f32 = mybir.dt.float32
    fp8 = mybir.dt.float8e4
    bf16 = mybir.dt.bfloat16
    ACTF = mybir.ActivationFunctionType

    NT = len(TS)
    FDtot = TS[-1][3] + TS[-1][2]
    NCH = FDtot // CHUNK
    assert FDtot % CHUNK == 0
    do_dma = variant != "v4nodma"
    do_pe = variant in ("v4", "v4nodma", "v4pe")
    do_act = variant in ("v4", "v4nodma", "v4act")

    nc = bacc.Bacc(
        "TRN2", target_bir_lowering=False, debug=False, num_devices=N_CORES
    )
    lg_d = nc.dram_tensor("lg", [P, FDtot], fp8, kind="ExternalInput")
    slab_d = nc.dram_tensor("slab", [P, FDtot], fp8, kind="ExternalInput")
    stats_d = nc.dram_tensor("stats", [P, NT + P], f32, kind="ExternalOutput")

    with tile.TileContext(nc) as tc, ExitStack() as ctx:
        iopool = ctx.enter_context(tc.tile_pool(name="io", bufs=2))
        spool = ctx.enter_context(tc.tile_pool(name="small", bufs=2))
        pspool = ctx.enter_context(tc.tile_pool(name="ps", bufs=2, space="PSUM"))
        fixed = ctx.enter_context(tc.tile_pool(name="fixed", bufs=1))
        scratch = fixed.tile([P, V], bf16)  # exp elementwise dump (reused)
        if not do_dma:
            lg_res = fixed.tile([P, FDtot], fp8)
            slab_res = fixed.tile([P, FDtot], fp8)
            nc.sync.dma_start(lg_res[:], lg_d.ap())
            nc.sync.dma_start(slab_res[:], slab_d.ap())

        def emit_pass():
            if do_dma:
                lg = iopool.tile([P, FDtot], fp8)
                slab = iopool.tile([P, FDtot], fp8)
                nc.sync.dma_start(lg[:], lg_d.ap())
                nc.sync.dma_start(slab[:], slab_d.ap())
            else:
                lg, slab = lg_res, slab_res
            stats = spool.tile([P, NT + P], f32)
            if do_pe:
                ps = pspool.tile([P, 512], f32)
                for c in range(NCH):
                    nc.tensor.matmul(
                        ps[:CHUNK, :CHUNK],
                        slab[:, CHUNK * c : CHUNK * (c + 1)],
                        lg[:, CHUNK * c : CHUNK * (c + 1)],
                        start=(c == 0),
                        stop=(c == NCH - 1),
                    )
                nc.vector.tensor_copy(stats[:, NT:], ps[:, :P])
            else:
                nc.any.memset(stats[:, NT:], 0.0)
            if do_act:
                for i, (n, s, fd, off) in enumerate(TS):
                    nc.scalar.activation(
                        out=scratch[:, :fd],
                        in_=lg[:, off : off + fd],
                        func=ACTF.Exp,
                        accum_out=stats[:, i : i + 1],
                    )
            else:
                nc.any.memset(stats[:, :NT], 1.0)
            # scalar (qAct) HWDGE ring: keeps this ACT-gated store out of
            # the sync ring's FIFO so it can't stall the next input stream
            nc.scalar.dma_start(stats_d.ap(), stats[:])

        if loop_reps:
            with tc.For_i(0, loop_reps, 1):
                for _ in range(reps):
                    emit_pass()
        else:
            for _ in range(reps):
                emit_pass()

    nc.compile()
    return nc


def _host_prep(ys, aligns, xlens):
    """Mirror of the reference's index math -> global non-blank frame list."""
    frame_mask = np.arange(T)[None, :] < xlens[:, None]
    a = np.where(frame_mask, aligns, BLANK)
    nonblank = a != BLANK
    shifted = np.concatenate([np.full((B, 1), BLANK, a.dtype), a[:, :-1]], axis=1)
    run_start = nonblank & (a != shifted)
    label_id = np.cumsum(run_start.astype(np.int64), axis=1) - 1
    lm = np.maximum(label_id, 0)
    n_exists = nonblank.sum(axis=1)

    bb, tt = np.nonzero(nonblank)
    w = 1.0 / (B * n_exists[bb].astype(np.float64))
    lmf = lm[bb, tt]
    y_t = np.asarray(ys)[bb, lmf]
    return bb, tt, lmf, y_t, w


def prepare(inputs: dict, variant: str = "v4"):
    """Host prep: index math, frame balancing, fp8 slab packing."""
    import ml_dtypes

    fp8np = ml_dtypes.float8_e4m3

    logits = np.asarray(inputs["logits"], dtype=np.float32)
    soft = np.asarray(inputs["soft_labels"], dtype=np.float32)
    ys = np.asarray(inputs["ys"])
    aligns = np.asarray(inputs["aligns"])
    xlens = np.asarray(inputs["xlens"])

    bb, tt, lmf, y_t, w = _host_prep(ys, aligns, xlens)
    NJ_tot = len(bb)
    ry = logits[bb, tt, y_t].astype(np.float64)

    q, r = divmod(NJ_tot, N_CORES)
    counts = [q + 1] * r + [q] * (N_CORES - r)
    njmax = q + (1 if r else 0)
    TS, FDtot = _geometry(njmax)

    key = (TS, variant)
    nc = _PROGRAM_CACHE.get(key)
    if nc is None:
        nc = _build_program(TS, variant=variant)
        _PROGRAM_CACHE[key] = nc

    starts = np.cumsum([0] + counts)
    in_maps, cores = [], []
    for c in range(N_CORES):
        s0, s1 = int(starts[c]), int(starts[c + 1])
        n = s1 - s0
        lg_rows = logits[bb[s0:s1], tt[s0:s1]]  # [n, V] f32
        soft_rows = soft[bb[s0:s1], lmf[s0:s1]]  # [n, V] f32
        slab_rows = (W_SOFT * soft_rows + (1.0 - W_SOFT) * A_R) * (
            w[s0:s1, None] * SCALE
        ).astype(np.float32)

        lg_flat = np.zeros((P, FDtot), np.float32)
        slab_flat = np.zeros((P, FDtot), np.float32)
        pos = 0
        for nf, s, fd, off in TS:
            m = min(nf, n - pos)
            if m > 0:
                lg_flat[: m * s, off : off + fd] = lg_rows[pos : pos + m].reshape(
                    m * s, fd
                )
                slab_flat[: m * s, off : off + fd] = slab_rows[
                    pos : pos + m
                ].reshape(m * s, fd)
            pos += nf
        in_maps.append(
            {"lg": lg_flat.astype(fp8np), "slab": slab_flat.astype(fp8np)}
        )
        cores.append(dict(n=n, w=w[s0:s1], ry=ry[s0:s1]))
    return nc, in_maps, cores, counts, TS


def combine(results, cores, NJ, TS, variant: str = "v4") -> np.float32:
    """Fold per-core [P, NT+P] stats into the scalar loss."""
    NT = len(TS)
    total = 0.0
    for c, ci in enumerate(cores):
        st = np.asarray(results[c]["stats"], dtype=np.float64)
        n = ci["n"]
        es = np.empty(n, np.float64)
        pos = 0
        for i, (nf, s, fd, off) in enumerate(TS):
            m = min(nf, n - pos)
            if m <= 0:
                break
            col = st[:, i]
            if s == 1:
                es[pos : pos + m] = col[:m]
            else:
                es[pos : pos + m] = col[: m * s].reshape(m, s).sum(axis=1)
            pos += nf
        lse = np.log(es)
        total += np.trace(st[:, NT:]) / SCALE
        total += float((ci["w"] * (C_Y * ci["ry"] - lse)).sum())
    return np.float32(-total)


def run(inputs: dict, variant: str = "v4", trace: bool = False, trace_cores=None):
    from concourse.bass_utils import run_bass_kernel_spmd

    nc, in_maps, cores, NJ, TS = prepare(inputs, variant)
    res = run_bass_kernel_spmd(
        nc,
        in_maps,
        list(range(N_CORES)),
        trace=trace,
        trace_cores=trace_cores,
    )
    loss = combine(res.results, cores, NJ, TS, variant)
    return loss, res


def kernel(**inputs) -> np.ndarray:
    loss, _ = run(inputs)
    return np.asarray(loss, dtype=np.float32)
